# revision 24
# baseline (speedup 1.0000x reference)
"""Trainium2 Bass kernel v3: Gaussian-RBF basis expansion + batched matmul.

Computes, for B=32 batches, N=65536 positions, DEG=32 basis functions,
D=8 output dims:
    basis[b,n,g] = exp(-(x[b,n] - c_g)^2 / (2*0.04))
    result[b,n,d] = sum_g basis[b,n,g] * weights[b,d,g]
and returns (result, zeros_like(result)).

Structure (v2 measured 113us: ScalarE had 10 psum-subtile ops/tile and the
2-bank psum ping-pong latency-coupled every broadcast matmul to it; 23us
startup on a monolithic 2MiB input DMA):
  * Quadratic broadcast: one K=80 indicator matmul per 512-subtile lands
    arg = 25*c_gg*x - 12.5*x^2 in PSUM per (degree,batch) partition (x and
    -12.5x^2 as bf16 hi/lo pairs; the 25*c coefficient split bf16-exact-hi
    + residual row).  A(x)=exp(-12.5x^2) is folded in; with the
    -12.5c_gg^2 activation bias, exp gives chunk-0 basis values directly.
  * Power ladder: chunks 1-3 are e_{j+1} = e_j * (T8/2) on VectorE (fp16
    tensor_tensor, 2x mode); T8 = exp(200h*x) is ONE global ScalarE exp,
    replicated 16->128 partitions by 8 small SBUF->SBUF DMAs per tile on
    the otherwise-idle DMA queues (v2's PE broadcast + ScalarE psum copies
    deleted).  2^j is folded into the bf16 weights so every e-chunk stays
    within fp16 range.
  * PSUM: 4 arg banks + 4 matmul banks; the 4-deep arg rotation lets the
    PE run arg broadcasts a full tile ahead of ScalarE's exps.
  * Pipeline phase: tile k's body runs Emm(k) on PE interleaved with
    arg(k+2); ScalarE runs evac(k-1) then e0(k+1); VectorE runs evac(k-1)
    then ladder(k+1) - so Emm(k) never waits on tile k's ladder chain.
  * Output evac split ScalarE (subtiles 0,1) / VectorE (2,3).
"""

import numpy as np
from contextlib import ExitStack

import concourse.bass as bass
from concourse import mybir
from concourse.bass_utils import run_bass_kernel_spmd

# Problem constants (hardcoded per harness contract)
B, D, N, DEG = 32, 8, 65536, 32
SCALE = 0.04
INV2S = 1.0 / (2.0 * SCALE)  # 12.5
NCORES = 8
NSH = N // NCORES  # 8192 positions per core

# Layout constants
T2 = 2048          # positions per pipeline tile
SUB = 512          # matmul moving-free-dim (one fp32 PSUM bank)
NQ = NSH // T2     # 4 position blocks
GB = 16            # batches per group
NG = B // GB       # 2 batch groups
PG = 8             # degrees per chunk
NCHUNK = DEG // PG # 4 chunks
NIT = NG * NQ      # 8 pipeline tiles
NSUB = T2 // SUB   # 4 sub-tiles per tile

H = 1.01 / 31.0
T8A = 2.0 * INV2S * PG * H  # 6.51612903...: T8 = exp(T8A * x)

FP = mybir.dt.float32
BF = mybir.dt.bfloat16
HF = mybir.dt.float16

_centers = np.linspace(0.0, 1.01, DEG).astype(np.float64)


def _build():
    nc = bass.Bass(
        "TRN2", target_bir_lowering=False, debug=False, num_devices=NCORES
    )
    xpk_d = nc.dram_tensor("xpk", [128, T2], FP, kind="ExternalInput")
    xc_d = nc.dram_tensor("xc", [96, NG * NSH], BF, kind="ExternalInput")
    lhsTa_d = nc.dram_tensor("lhsTa", [96, 128], BF, kind="ExternalInput")
    lhsTw_d = nc.dram_tensor(
        "lhsTw", [128, NG, NCHUNK, 128], BF, kind="ExternalInput"
    )
    ebias_d = nc.dram_tensor("ebias", [128, 2], FP, kind="ExternalInput")
    out_d = nc.dram_tensor("out", [NG, 128, NSH], HF, kind="ExternalOutput")

    EXP = mybir.ActivationFunctionType.Exp
    QH = NSH // 2  # xc DMA quarter (per-group half)

    with ExitStack() as ctx:
        en = ctx.enter_context
        # --- SBUF ---------------------------------------------------------
        xpk = en(nc.sbuf_tensor("xpk_sb", [128, T2], FP)).ap()
        xc = en(nc.sbuf_tensor("xc_sb", [96, NG * NSH], BF)).ap()
        lhsTa = en(nc.sbuf_tensor("lhsTa_sb", [96, 128], BF)).ap()
        lhsTw = en(nc.sbuf_tensor("lhsTw_sb", [128, NG, NCHUNK, 128], BF)).ap()
        ebias = en(nc.sbuf_tensor("ebias_sb", [128, 2], FP)).ap()
        dumm = en(nc.sbuf_tensor("dumm_sb", [128, 1], FP)).ap()
        t8pk = en(nc.sbuf_tensor("t8pk_sb", [128, T2], HF)).ap()
        t8r = en(nc.sbuf_tensor("t8r_sb", [128, NIT * T2], HF)).ap()
        e = [
            [en(nc.sbuf_tensor(f"e{i}_{j}", [128, T2], HF)).ap()
             for j in range(NCHUNK)]
            for i in range(3)
        ]
        osb = [en(nc.sbuf_tensor(f"osb{i}", [128, T2], HF)).ap() for i in range(3)]
        # --- PSUM: 4 arg banks + 4 matmul banks --------------------------
        pa = [en(nc.psum_tensor(f"pa{s}", [128, SUB], FP)).ap() for s in range(NSUB)]
        pm = [en(nc.psum_tensor(f"pm{s}", [128, SUB], FP)).ap() for s in range(NSUB)]
        # --- semaphores ---------------------------------------------------
        s_xcp = [en(nc.semaphore(f"s_xcp{i}")) for i in range(4)]  # xc quarters
        s_xpk = en(nc.semaphore("s_xpk"))  # xpk landed
        s_lwa = en(nc.semaphore("s_lwa"))  # lhsTa landed
        s_lww = en(nc.semaphore("s_lww"))  # lhsTw landed
        s_leb = en(nc.semaphore("s_leb"))  # ebias landed
        s_t8g = en(nc.semaphore("s_t8g"))  # global T8 exp done
        s_t8p = [en(nc.semaphore(f"s_t8p{i}")) for i in range(2)]  # t8 repl (parity)
        s_arg = en(nc.semaphore("s_arg"))  # PE arg-bcast subtile (+1)
        s_e0 = en(nc.semaphore("s_e0"))    # ScalarE e0 subtile (+1)
        s_lad = en(nc.semaphore("s_lad"))  # DVE ladder chunk (+1)
        s_mm = en(nc.semaphore("s_mm"))    # PE E-mm subtile (+1 at j=3)
        s_eva = en(nc.semaphore("s_eva"))  # ScalarE evac subtiles 0,1 (+1)
        s_evb = en(nc.semaphore("s_evb"))  # DVE evac subtiles 2,3 (+1)
        s_out = [en(nc.semaphore(f"s_out{i}")) for i in range(3)]  # out DMA done, by osb slot

        with nc.Block() as block:

            @block.sync
            def _(sync):
                sync.dma_start(out=lhsTa, in_=lhsTa_d.ap()).then_inc(s_lwa, 16)
                for i in range(4):
                    cs = slice(i * QH, (i + 1) * QH)
                    sync.dma_start(out=xc[:, cs], in_=xc_d.ap()[:, cs]
                                   ).then_inc(s_xcp[i], 16)
                sync.dma_start(out=lhsTw, in_=lhsTw_d.ap()).then_inc(s_lww, 16)
                # T8 replication: 16 -> 128 partitions, 8 small DMAs per
                # tile, interleaved with the output DMAs so out(0) is not
                # stuck behind 64 DMA issues on this queue
                def t8rep(it):
                    g, q = divmod(it, NQ)
                    r0 = 32 * q + GB * g
                    # same-parity batches serialized (the parity sem counts
                    # completions of ANY in-flight DMA of that parity), two
                    # parities overlap in flight
                    if it >= 2:
                        sync.wait_ge(s_t8p[it % 2], 128 * (it // 2))
                    for k in range(8):
                        sync.dma_start(
                            out=t8r[GB * k : GB * (k + 1),
                                    T2 * it : T2 * (it + 1)],
                            in_=t8pk[r0 : r0 + GB, :],
                        ).then_inc(s_t8p[it % 2], 16)

                sync.wait_ge(s_t8g, 1)
                t8rep(0)
                t8rep(1)
                t8rep(2)
                for it in range(NIT):
                    if it + 3 < NIT:
                        t8rep(it + 3)
                    g, q = divmod(it, NQ)
                    sync.wait_ge(s_eva, 2 * (it + 1))
                    sync.wait_ge(s_evb, 2 * (it + 1))
                    sync.dma_start(
                        out=out_d.ap()[g, :, T2 * q : T2 * (q + 1)],
                        in_=osb[it % 3],
                    ).then_inc(s_out[it % 3], 16)

            @block.scalar
            def _(scalar):
                # dummy exp triggers the ACT table load while xpk streams
                scalar.activation(dumm, dumm, EXP, scale=0.0)
                scalar.dma_start(out=ebias, in_=ebias_d.ap()).then_inc(s_leb, 16)
                scalar.dma_start(out=xpk[:, : T2 // 2],
                                 in_=xpk_d.ap()[:, : T2 // 2]
                                 ).then_inc(s_xpk, 16)
                scalar.dma_start(out=xpk[:, T2 // 2 :],
                                 in_=xpk_d.ap()[:, T2 // 2 :]
                                 ).then_inc(s_xpk, 16)
                scalar.wait_ge(s_xpk, 32)
                # bias ln(1/2): t8pk = exp(T8A*x)/2, the ladder's per-step
                # halving (2^j is folded into the matmul weights)
                scalar.wait_ge(s_leb, 16)
                scalar.activation(
                    t8pk, xpk, EXP, scale=T8A, bias=ebias[:, 1:2]
                ).then_inc(s_t8g, 1)

                def ev_s(itp, s):
                    bo = itp % 3
                    scalar.wait_ge(s_mm, NSUB * itp + s + 1)
                    if itp >= 3 and s == 0:
                        # osb WAR: out(itp-3) fully drained (same slot)
                        scalar.wait_ge(s_out[itp % 3], 16 * (itp // 3))
                    scalar.copy(
                        osb[bo][:, SUB * s : SUB * (s + 1)], pm[s]
                    ).then_inc(s_eva, 1)

                def e0s(it, s):
                    bi = it % 3
                    scalar.wait_ge(s_arg, NSUB * it + s + 1)
                    if it >= 3 and s == 0:
                        scalar.wait_ge(s_mm, NSUB * (it - 2))  # e0 WAR
                    if it == 0 and s == 0:
                        scalar.wait_ge(s_leb, 16)
                    scalar.activation(
                        e[bi][0][:, SUB * s : SUB * (s + 1)], pa[s],
                        EXP, scale=1.0, bias=ebias[:, 0:1],
                    ).then_inc(s_e0, 1)

                for s in range(NSUB):
                    e0s(0, s)
                for k in range(NIT):
                    if k >= 1:
                        ev_s(k - 1, 0)
                    if k + 1 < NIT:
                        e0s(k + 1, 0)
                    if k >= 1:
                        ev_s(k - 1, 1)
                    if k + 1 < NIT:
                        for s in range(1, NSUB):
                            e0s(k + 1, s)
                ev_s(NIT - 1, 0)
                ev_s(NIT - 1, 1)

            @block.vector
            def _(vector):
                def ev_v(itp):
                    bo = itp % 3
                    for s in (2, 3):
                        vector.wait_ge(s_mm, NSUB * itp + s + 1)
                        if itp >= 3 and s == 2:
                            # osb WAR: out(itp-3) fully drained (same slot)
                            vector.wait_ge(s_out[itp % 3], 16 * (itp // 3))
                        vector.tensor_copy(
                            osb[bo][:, SUB * s : SUB * (s + 1)], pm[s]
                        ).then_inc(s_evb, 1)

                def ladder(it):
                    bi = it % 3
                    t8v = t8r[:, T2 * it : T2 * (it + 1)]
                    vector.wait_ge(s_e0, NSUB * (it + 1))
                    vector.wait_ge(s_t8p[it % 2], 128 * (it // 2 + 1))
                    if it >= 3:
                        vector.wait_ge(s_mm, NSUB * (it - 2))  # e[1..3] WAR
                    vector.tensor_mul(e[bi][1], e[bi][0], t8v).then_inc(s_lad, 1)
                    vector.tensor_mul(e[bi][2], e[bi][1], t8v).then_inc(s_lad, 1)
                    vector.tensor_mul(e[bi][3], e[bi][2], t8v).then_inc(s_lad, 1)

                ladder(0)
                for k in range(NIT):
                    if k >= 1:
                        ev_v(k - 1)
                    if k + 1 < NIT:
                        ladder(k + 1)
                ev_v(NIT - 1)

            @block.tensor
            def _(tensor):
                # warm-up: keep the PE array busy while inputs stream so the
                # HAM clock-gate reaches 8/8 before the real stream begins
                # (reads uninitialized SBUF, writes pm[0]; both harmless -
                # the first real accumulation starts with start=True)
                def warm(n):
                    for _ in range(n):
                        tensor.matmul(
                            pm[0][:, 0:256], osb[1][:, 0:128], osb[0][:, 0:256],
                            start=True, stop=True, skip_group_check=True,
                        )

                def bc_arg(it, s):
                    g, q = divmod(it, NQ)
                    if it == 0 and s == 0:
                        tensor.wait_ge(s_lwa, 16)
                    if it % 2 == 0 and s == 0:
                        tensor.wait_ge(s_xcp[it // 2], 16)
                    if it >= 1:
                        # pa[s] WAR: previous tile's e0 subtile s consumed
                        tensor.wait_ge(s_e0, NSUB * (it - 1) + s + 1)
                    c0 = g * NSH + q * T2 + SUB * s
                    tensor.matmul(
                        pa[s], lhsTa, xc[:, c0 : c0 + SUB],
                        start=True, stop=True, skip_group_check=True,
                    ).then_inc(s_arg, 1)

                def emm_grp(it, s):
                    bi = it % 3
                    g = it // NQ
                    for j in range(NCHUNK):
                        if j == 0:
                            if it == 0 and s == 0:
                                tensor.wait_ge(s_lww, 16)
                            if s == 0:
                                # subsumes e0(it) completion for all subtiles
                                tensor.wait_ge(s_lad, 3 * it + 1)
                            if it >= 1:
                                if s < 2:
                                    tensor.wait_ge(s_eva, 2 * (it - 1) + s + 1)
                                else:
                                    tensor.wait_ge(s_evb, 2 * (it - 1) + s - 1)
                        elif s == 0 and j >= 2:
                            tensor.wait_ge(s_lad, 3 * it + j)
                        mm = tensor.matmul(
                            pm[s], lhsTw[:, g, j, :],
                            e[bi][j][:, SUB * s : SUB * (s + 1)],
                            start=(j == 0), stop=(j == NCHUNK - 1),
                            skip_group_check=True,
                        )
                        if j == NCHUNK - 1:
                            mm.then_inc(s_mm, 1)

                warm(25)
                for s in range(NSUB):
                    bc_arg(0, s)
                warm(12)
                for s in range(NSUB):
                    bc_arg(1, s)
                warm(12)
                for k in range(NIT):
                    for s in range(NSUB):
                        emm_grp(k, s)
                        if k + 2 < NIT:
                            bc_arg(k + 2, s)
    return nc


def _host_inputs(weights, positions):
    """Per-core in_maps: bit-level packing, hi/lo splits and x^2 only."""
    import ml_dtypes

    bf = ml_dtypes.bfloat16
    w = np.ascontiguousarray(np.asarray(weights, dtype=np.float32))
    x = np.ascontiguousarray(np.asarray(positions, dtype=np.float32))
    cent = _centers

    ggm = np.arange(128) // GB  # degree-in-chunk of partition/column m
    bm = np.arange(128) % GB    # batch-in-group of partition/column m

    # lhsTa [128,128]: rows 0-15 x_hi, 16-31 x_lo, 32-47 x_hi(dup),
    # 48-63 msq_hi, 64-79 msq_lo; coefficient 25*c split bf16-exact hi
    # plus residual on the duplicated x_hi rows
    coef = 2.0 * INV2S * cent[:PG]
    chi = np.float32(coef).astype(bf).astype(np.float64)
    clo = np.float32(coef - chi)
    chif = np.float32(chi)
    # 96 rows: K must be a multiple of 32 (PE row-group granularity);
    # rows 80-95 are explicit zeros in both weights and moving data
    lhsTa = np.zeros((96, 128), np.float32)
    for k in range(GB):
        sel = bm == k
        lhsTa[k, sel] = chif[ggm[sel]]
        lhsTa[GB + k, sel] = chif[ggm[sel]]
        lhsTa[2 * GB + k, sel] = clo[ggm[sel]]
        lhsTa[3 * GB + k, sel] = 1.0
        lhsTa[4 * GB + k, sel] = 1.0
    lhsTa = lhsTa.astype(bf)

    # per-partition activation bias: col0 = -12.5*c_gg^2, col1 = ln(1/2)
    ebias = np.zeros((128, 2), np.float32)
    ebias[:, 0] = np.float32(-INV2S * cent[:PG] ** 2)[ggm]
    ebias[:, 1] = np.float32(np.log(0.5))
    ebias = np.ascontiguousarray(ebias)

    # E-matmul weights: ladder rescale exp(-12.5(c_{8j+gg}^2-c_gg^2)) and
    # the 2^j compensation for the T8/2 ladder steps folded in
    jj = np.arange(NCHUNK)[:, None]
    gg = np.arange(PG)[None, :]
    fac = np.exp(-INV2S * (cent[PG * jj + gg] ** 2 - cent[gg] ** 2))
    fac = fac * (2.0 ** np.arange(NCHUNK))[:, None]
    w4 = w.reshape(NG, GB, D, NCHUNK, PG).astype(np.float64)
    w4 = w4 * fac[None, None, None, :, :]
    eye = np.eye(GB)
    lhsTw = np.einsum("gbdjh,cb->hcgjdb", w4, eye)  # [gg, b', g, j, d, b]
    lhsTw = np.ascontiguousarray(
        lhsTw.reshape(128, NG, NCHUNK, 128).astype(bf)
    )

    in_maps = []
    for ci in range(NCORES):
        xs = x[:, ci * NSH : (ci + 1) * NSH]  # [32, NSH]
        xpk = np.ascontiguousarray(
            xs.reshape(B, NQ, T2).transpose(1, 0, 2).reshape(128, T2)
        )
        xh = xs.astype(bf)
        xl = (xs - xh.astype(np.float32)).astype(bf)
        msq = (-INV2S * (xs.astype(np.float64) ** 2)).astype(np.float32)
        msqh = msq.astype(bf)
        msql = (msq - msqh.astype(np.float32)).astype(bf)
        xc = np.zeros((96, NG * NSH), bf)
        for g in range(NG):
            blk = slice(g * NSH, (g + 1) * NSH)
            rows = slice(GB * g, GB * (g + 1))
            xc[0:GB, blk] = xh[rows]
            xc[GB : 2 * GB, blk] = xl[rows]
            xc[2 * GB : 3 * GB, blk] = xh[rows]
            xc[3 * GB : 4 * GB, blk] = msqh[rows]
            xc[4 * GB : 5 * GB, blk] = msql[rows]
        in_maps.append(
            {
                "xpk": xpk,
                "xc": np.ascontiguousarray(xc),
                "lhsTa": lhsTa,
                "lhsTw": lhsTw,
                "ebias": ebias,
            }
        )
    return in_maps


def _gather(results):
    """[NG, 128, NSH] per core, rows m=d*16+b  ->  full [B, N, D]."""
    outs = []
    for r in results:
        o = r["out"].astype(np.float32).reshape(NG, D, GB, NSH)  # [g, d, b, n]
        outs.append(o.transpose(0, 2, 3, 1).reshape(B, NSH, D))  # [b, n, d]
    full = np.concatenate(outs, axis=1)  # [B, N, D]
    return np.ascontiguousarray(full)


_NC_CACHE = {}


def run(inputs, trace=False, **trace_kwargs):
    """Builds (cached), runs on 8 cores, returns ((result, zeros), results)."""
    key = ("v3",)
    if key not in _NC_CACHE:
        _NC_CACHE[key] = _build()
    nc = _NC_CACHE[key]
    in_maps = _host_inputs(inputs["weights"], inputs["positions"])
    br = run_bass_kernel_spmd(
        nc, in_maps, list(range(NCORES)), trace=trace, **trace_kwargs
    )
    result = _gather(br.results)
    return (result, np.zeros_like(result)), br


def kernel(weights, weights_std, positions):
    out, _ = run(
        {"weights": weights, "weights_std": weights_std, "positions": positions}
    )
    return out


# revision 25
# speedup vs baseline: 1.0020x; 1.0020x over previous
"""Trainium2 Bass kernel v3: Gaussian-RBF basis expansion + batched matmul.

Computes, for B=32 batches, N=65536 positions, DEG=32 basis functions,
D=8 output dims:
    basis[b,n,g] = exp(-(x[b,n] - c_g)^2 / (2*0.04))
    result[b,n,d] = sum_g basis[b,n,g] * weights[b,d,g]
and returns (result, zeros_like(result)).

Structure (v2 measured 113us: ScalarE had 10 psum-subtile ops/tile and the
2-bank psum ping-pong latency-coupled every broadcast matmul to it; 23us
startup on a monolithic 2MiB input DMA):
  * Quadratic broadcast: one K=80 indicator matmul per 512-subtile lands
    arg = 25*c_gg*x - 12.5*x^2 in PSUM per (degree,batch) partition (x and
    -12.5x^2 as bf16 hi/lo pairs; the 25*c coefficient split bf16-exact-hi
    + residual row).  A(x)=exp(-12.5x^2) is folded in; with the
    -12.5c_gg^2 activation bias, exp gives chunk-0 basis values directly.
  * Power ladder: chunks 1-3 are e_{j+1} = e_j * (T8/2) on VectorE (fp16
    tensor_tensor, 2x mode); T8 = exp(200h*x) is ONE global ScalarE exp,
    replicated 16->128 partitions by 8 small SBUF->SBUF DMAs per tile on
    the otherwise-idle DMA queues (v2's PE broadcast + ScalarE psum copies
    deleted).  2^j is folded into the bf16 weights so every e-chunk stays
    within fp16 range.
  * PSUM: 4 arg banks + 4 matmul banks; the 4-deep arg rotation lets the
    PE run arg broadcasts a full tile ahead of ScalarE's exps.
  * Pipeline phase: tile k's body runs Emm(k) on PE interleaved with
    arg(k+2); ScalarE runs evac(k-1) then e0(k+1); VectorE runs evac(k-1)
    then ladder(k+1) - so Emm(k) never waits on tile k's ladder chain.
  * Output evac split ScalarE (subtiles 0,1) / VectorE (2,3).
"""

import numpy as np
from contextlib import ExitStack

import concourse.bass as bass
from concourse import mybir
from concourse.bass_utils import run_bass_kernel_spmd

# Problem constants (hardcoded per harness contract)
B, D, N, DEG = 32, 8, 65536, 32
SCALE = 0.04
INV2S = 1.0 / (2.0 * SCALE)  # 12.5
NCORES = 8
NSH = N // NCORES  # 8192 positions per core

# Layout constants
T2 = 2048          # positions per pipeline tile
SUB = 512          # matmul moving-free-dim (one fp32 PSUM bank)
NQ = NSH // T2     # 4 position blocks
GB = 16            # batches per group
NG = B // GB       # 2 batch groups
PG = 8             # degrees per chunk
NCHUNK = DEG // PG # 4 chunks
NIT = NG * NQ      # 8 pipeline tiles
NSUB = T2 // SUB   # 4 sub-tiles per tile

H = 1.01 / 31.0
T8A = 2.0 * INV2S * PG * H  # 6.51612903...: T8 = exp(T8A * x)

FP = mybir.dt.float32
BF = mybir.dt.bfloat16
HF = mybir.dt.float16

_centers = np.linspace(0.0, 1.01, DEG).astype(np.float64)


def _build():
    nc = bass.Bass(
        "TRN2", target_bir_lowering=False, debug=False, num_devices=NCORES
    )
    xpk_d = nc.dram_tensor("xpk", [128, T2], FP, kind="ExternalInput")
    xc_d = nc.dram_tensor("xc", [96, NG * NSH], BF, kind="ExternalInput")
    lhsTa_d = nc.dram_tensor("lhsTa", [96, 128], BF, kind="ExternalInput")
    lhsTw_d = nc.dram_tensor(
        "lhsTw", [128, NG, NCHUNK, 128], BF, kind="ExternalInput"
    )
    ebias_d = nc.dram_tensor("ebias", [128, 2], FP, kind="ExternalInput")
    out_d = nc.dram_tensor("out", [NG, 128, NSH], HF, kind="ExternalOutput")

    EXP = mybir.ActivationFunctionType.Exp
    QH = NSH // 2  # xc DMA quarter (per-group half)

    with ExitStack() as ctx:
        en = ctx.enter_context
        # --- SBUF ---------------------------------------------------------
        xpk = en(nc.sbuf_tensor("xpk_sb", [128, T2], FP)).ap()
        xc = en(nc.sbuf_tensor("xc_sb", [96, NG * NSH], BF)).ap()
        lhsTa = en(nc.sbuf_tensor("lhsTa_sb", [96, 128], BF)).ap()
        lhsTw = en(nc.sbuf_tensor("lhsTw_sb", [128, NG, NCHUNK, 128], BF)).ap()
        ebias = en(nc.sbuf_tensor("ebias_sb", [128, 2], FP)).ap()
        dumm = en(nc.sbuf_tensor("dumm_sb", [128, 1], FP)).ap()
        t8pk = en(nc.sbuf_tensor("t8pk_sb", [128, T2], HF)).ap()
        t8r = en(nc.sbuf_tensor("t8r_sb", [128, NIT * T2], HF)).ap()
        e = [
            [en(nc.sbuf_tensor(f"e{i}_{j}", [128, T2], HF)).ap()
             for j in range(NCHUNK)]
            for i in range(3)
        ]
        osb = [en(nc.sbuf_tensor(f"osb{i}", [128, T2], HF)).ap() for i in range(3)]
        # --- PSUM: 4 arg banks + 4 matmul banks --------------------------
        pa = [en(nc.psum_tensor(f"pa{s}", [128, SUB], FP)).ap() for s in range(NSUB)]
        pm = [en(nc.psum_tensor(f"pm{s}", [128, SUB], FP)).ap() for s in range(NSUB)]
        # --- semaphores ---------------------------------------------------
        s_xcp = [en(nc.semaphore(f"s_xcp{i}")) for i in range(4)]  # xc quarters
        s_xpk = en(nc.semaphore("s_xpk"))  # xpk landed
        s_lwa = en(nc.semaphore("s_lwa"))  # lhsTa landed
        s_lww = en(nc.semaphore("s_lww"))  # lhsTw landed
        s_leb = en(nc.semaphore("s_leb"))  # ebias landed
        s_t8g = en(nc.semaphore("s_t8g"))  # global T8 exp done
        s_t8p = [en(nc.semaphore(f"s_t8p{i}")) for i in range(2)]  # t8 repl (parity)
        s_arg = en(nc.semaphore("s_arg"))  # PE arg-bcast subtile (+1)
        s_e0 = en(nc.semaphore("s_e0"))    # ScalarE e0 subtile (+1)
        s_lad = en(nc.semaphore("s_lad"))  # DVE ladder chunk (+1)
        s_mm = en(nc.semaphore("s_mm"))    # PE E-mm subtile (+1 at j=3)
        s_eva = en(nc.semaphore("s_eva"))  # ScalarE evac subtiles 0,1 (+1)
        s_evb = en(nc.semaphore("s_evb"))  # DVE evac subtiles 2,3 (+1)
        s_out = [en(nc.semaphore(f"s_out{i}")) for i in range(3)]  # out DMA done, by osb slot

        with nc.Block() as block:

            @block.sync
            def _(sync):
                sync.dma_start(out=lhsTa, in_=lhsTa_d.ap()).then_inc(s_lwa, 16)
                # quarter 0 first and alone: it gates the whole tile-0 chain;
                # quarters 1-3 are not needed until tiles 2/4/6 and would
                # steal DMA bandwidth from it
                sync.dma_start(out=xc[:, 0:QH], in_=xc_d.ap()[:, 0:QH]
                               ).then_inc(s_xcp[0], 16)
                sync.dma_start(out=lhsTw, in_=lhsTw_d.ap()).then_inc(s_lww, 16)
                # T8 replication: 16 -> 128 partitions, 8 small DMAs per
                # tile, interleaved with the output DMAs so out(0) is not
                # stuck behind 64 DMA issues on this queue
                def t8rep(it):
                    g, q = divmod(it, NQ)
                    r0 = 32 * q + GB * g
                    # same-parity batches serialized (the parity sem counts
                    # completions of ANY in-flight DMA of that parity), two
                    # parities overlap in flight
                    if it >= 2:
                        sync.wait_ge(s_t8p[it % 2], 128 * (it // 2))
                    for k in range(8):
                        sync.dma_start(
                            out=t8r[GB * k : GB * (k + 1),
                                    T2 * it : T2 * (it + 1)],
                            in_=t8pk[r0 : r0 + GB, :],
                        ).then_inc(s_t8p[it % 2], 16)

                sync.wait_ge(s_t8g, 1)
                t8rep(0)
                t8rep(1)
                for i in range(1, 4):
                    cs = slice(i * QH, (i + 1) * QH)
                    sync.dma_start(out=xc[:, cs], in_=xc_d.ap()[:, cs]
                                   ).then_inc(s_xcp[i], 16)
                t8rep(2)
                for it in range(NIT):
                    if it + 3 < NIT:
                        t8rep(it + 3)
                    g, q = divmod(it, NQ)
                    sync.wait_ge(s_eva, 2 * (it + 1))
                    sync.wait_ge(s_evb, 2 * (it + 1))
                    sync.dma_start(
                        out=out_d.ap()[g, :, T2 * q : T2 * (q + 1)],
                        in_=osb[it % 3],
                    ).then_inc(s_out[it % 3], 16)

            @block.scalar
            def _(scalar):
                # dummy exp triggers the ACT table load while xpk streams
                scalar.activation(dumm, dumm, EXP, scale=0.0)
                scalar.dma_start(out=ebias, in_=ebias_d.ap()).then_inc(s_leb, 16)
                scalar.dma_start(out=xpk[:, : T2 // 2],
                                 in_=xpk_d.ap()[:, : T2 // 2]
                                 ).then_inc(s_xpk, 16)
                scalar.dma_start(out=xpk[:, T2 // 2 :],
                                 in_=xpk_d.ap()[:, T2 // 2 :]
                                 ).then_inc(s_xpk, 16)
                scalar.wait_ge(s_xpk, 32)
                # bias ln(1/2): t8pk = exp(T8A*x)/2, the ladder's per-step
                # halving (2^j is folded into the matmul weights)
                scalar.wait_ge(s_leb, 16)
                scalar.activation(
                    t8pk, xpk, EXP, scale=T8A, bias=ebias[:, 1:2]
                ).then_inc(s_t8g, 1)

                def ev_s(itp, s):
                    bo = itp % 3
                    scalar.wait_ge(s_mm, NSUB * itp + s + 1)
                    if itp >= 3 and s == 0:
                        # osb WAR: out(itp-3) fully drained (same slot)
                        scalar.wait_ge(s_out[itp % 3], 16 * (itp // 3))
                    scalar.copy(
                        osb[bo][:, SUB * s : SUB * (s + 1)], pm[s]
                    ).then_inc(s_eva, 1)

                def e0s(it, s):
                    bi = it % 3
                    scalar.wait_ge(s_arg, NSUB * it + s + 1)
                    if it >= 3 and s == 0:
                        scalar.wait_ge(s_mm, NSUB * (it - 2))  # e0 WAR
                    if it == 0 and s == 0:
                        scalar.wait_ge(s_leb, 16)
                    scalar.activation(
                        e[bi][0][:, SUB * s : SUB * (s + 1)], pa[s],
                        EXP, scale=1.0, bias=ebias[:, 0:1],
                    ).then_inc(s_e0, 1)

                for s in range(NSUB):
                    e0s(0, s)
                for k in range(NIT):
                    if k >= 1:
                        ev_s(k - 1, 0)
                    if k + 1 < NIT:
                        e0s(k + 1, 0)
                    if k >= 1:
                        ev_s(k - 1, 1)
                    if k + 1 < NIT:
                        for s in range(1, NSUB):
                            e0s(k + 1, s)
                ev_s(NIT - 1, 0)
                ev_s(NIT - 1, 1)

            @block.vector
            def _(vector):
                def ev_v(itp):
                    bo = itp % 3
                    for s in (2, 3):
                        vector.wait_ge(s_mm, NSUB * itp + s + 1)
                        if itp >= 3 and s == 2:
                            # osb WAR: out(itp-3) fully drained (same slot)
                            vector.wait_ge(s_out[itp % 3], 16 * (itp // 3))
                        vector.tensor_copy(
                            osb[bo][:, SUB * s : SUB * (s + 1)], pm[s]
                        ).then_inc(s_evb, 1)

                def ladder(it):
                    bi = it % 3
                    t8v = t8r[:, T2 * it : T2 * (it + 1)]
                    vector.wait_ge(s_e0, NSUB * (it + 1))
                    vector.wait_ge(s_t8p[it % 2], 128 * (it // 2 + 1))
                    if it >= 3:
                        vector.wait_ge(s_mm, NSUB * (it - 2))  # e[1..3] WAR
                    vector.tensor_mul(e[bi][1], e[bi][0], t8v).then_inc(s_lad, 1)
                    vector.tensor_mul(e[bi][2], e[bi][1], t8v).then_inc(s_lad, 1)
                    vector.tensor_mul(e[bi][3], e[bi][2], t8v).then_inc(s_lad, 1)

                ladder(0)
                for k in range(NIT):
                    if k >= 1:
                        ev_v(k - 1)
                    if k + 1 < NIT:
                        ladder(k + 1)
                ev_v(NIT - 1)

            @block.tensor
            def _(tensor):
                # warm-up: keep the PE array busy while inputs stream so the
                # HAM clock-gate reaches 8/8 before the real stream begins
                # (reads uninitialized SBUF, writes pm[0]; both harmless -
                # the first real accumulation starts with start=True)
                def warm(n):
                    for _ in range(n):
                        tensor.matmul(
                            pm[0][:, 0:256], osb[1][:, 0:128], osb[0][:, 0:256],
                            start=True, stop=True, skip_group_check=True,
                        )

                def bc_arg(it, s):
                    g, q = divmod(it, NQ)
                    if it == 0 and s == 0:
                        tensor.wait_ge(s_lwa, 16)
                    if it % 2 == 0 and s == 0:
                        tensor.wait_ge(s_xcp[it // 2], 16)
                    if it >= 1:
                        # pa[s] WAR: previous tile's e0 subtile s consumed
                        tensor.wait_ge(s_e0, NSUB * (it - 1) + s + 1)
                    c0 = g * NSH + q * T2 + SUB * s
                    tensor.matmul(
                        pa[s], lhsTa, xc[:, c0 : c0 + SUB],
                        start=True, stop=True, skip_group_check=True,
                    ).then_inc(s_arg, 1)

                def emm_grp(it, s):
                    bi = it % 3
                    g = it // NQ
                    for j in range(NCHUNK):
                        if j == 0:
                            if it == 0 and s == 0:
                                tensor.wait_ge(s_lww, 16)
                            if s == 0:
                                # subsumes e0(it) completion for all subtiles
                                tensor.wait_ge(s_lad, 3 * it + 1)
                            if it >= 1:
                                if s < 2:
                                    tensor.wait_ge(s_eva, 2 * (it - 1) + s + 1)
                                else:
                                    tensor.wait_ge(s_evb, 2 * (it - 1) + s - 1)
                        elif s == 0 and j >= 2:
                            tensor.wait_ge(s_lad, 3 * it + j)
                        mm = tensor.matmul(
                            pm[s], lhsTw[:, g, j, :],
                            e[bi][j][:, SUB * s : SUB * (s + 1)],
                            start=(j == 0), stop=(j == NCHUNK - 1),
                            skip_group_check=True,
                        )
                        if j == NCHUNK - 1:
                            mm.then_inc(s_mm, 1)

                warm(25)
                for s in range(NSUB):
                    bc_arg(0, s)
                warm(12)
                for s in range(NSUB):
                    bc_arg(1, s)
                warm(12)
                for k in range(NIT):
                    for s in range(NSUB):
                        emm_grp(k, s)
                        if k + 2 < NIT:
                            bc_arg(k + 2, s)
    return nc


def _host_inputs(weights, positions):
    """Per-core in_maps: bit-level packing, hi/lo splits and x^2 only."""
    import ml_dtypes

    bf = ml_dtypes.bfloat16
    w = np.ascontiguousarray(np.asarray(weights, dtype=np.float32))
    x = np.ascontiguousarray(np.asarray(positions, dtype=np.float32))
    cent = _centers

    ggm = np.arange(128) // GB  # degree-in-chunk of partition/column m
    bm = np.arange(128) % GB    # batch-in-group of partition/column m

    # lhsTa [128,128]: rows 0-15 x_hi, 16-31 x_lo, 32-47 x_hi(dup),
    # 48-63 msq_hi, 64-79 msq_lo; coefficient 25*c split bf16-exact hi
    # plus residual on the duplicated x_hi rows
    coef = 2.0 * INV2S * cent[:PG]
    chi = np.float32(coef).astype(bf).astype(np.float64)
    clo = np.float32(coef - chi)
    chif = np.float32(chi)
    # 96 rows: K must be a multiple of 32 (PE row-group granularity);
    # rows 80-95 are explicit zeros in both weights and moving data
    lhsTa = np.zeros((96, 128), np.float32)
    for k in range(GB):
        sel = bm == k
        lhsTa[k, sel] = chif[ggm[sel]]
        lhsTa[GB + k, sel] = chif[ggm[sel]]
        lhsTa[2 * GB + k, sel] = clo[ggm[sel]]
        lhsTa[3 * GB + k, sel] = 1.0
        lhsTa[4 * GB + k, sel] = 1.0
    lhsTa = lhsTa.astype(bf)

    # per-partition activation bias: col0 = -12.5*c_gg^2, col1 = ln(1/2)
    ebias = np.zeros((128, 2), np.float32)
    ebias[:, 0] = np.float32(-INV2S * cent[:PG] ** 2)[ggm]
    ebias[:, 1] = np.float32(np.log(0.5))
    ebias = np.ascontiguousarray(ebias)

    # E-matmul weights: ladder rescale exp(-12.5(c_{8j+gg}^2-c_gg^2)) and
    # the 2^j compensation for the T8/2 ladder steps folded in
    jj = np.arange(NCHUNK)[:, None]
    gg = np.arange(PG)[None, :]
    fac = np.exp(-INV2S * (cent[PG * jj + gg] ** 2 - cent[gg] ** 2))
    fac = fac * (2.0 ** np.arange(NCHUNK))[:, None]
    w4 = w.reshape(NG, GB, D, NCHUNK, PG).astype(np.float64)
    w4 = w4 * fac[None, None, None, :, :]
    eye = np.eye(GB)
    lhsTw = np.einsum("gbdjh,cb->hcgjdb", w4, eye)  # [gg, b', g, j, d, b]
    lhsTw = np.ascontiguousarray(
        lhsTw.reshape(128, NG, NCHUNK, 128).astype(bf)
    )

    in_maps = []
    for ci in range(NCORES):
        xs = x[:, ci * NSH : (ci + 1) * NSH]  # [32, NSH]
        xpk = np.ascontiguousarray(
            xs.reshape(B, NQ, T2).transpose(1, 0, 2).reshape(128, T2)
        )
        xh = xs.astype(bf)
        xl = (xs - xh.astype(np.float32)).astype(bf)
        msq = (-INV2S * (xs.astype(np.float64) ** 2)).astype(np.float32)
        msqh = msq.astype(bf)
        msql = (msq - msqh.astype(np.float32)).astype(bf)
        xc = np.zeros((96, NG * NSH), bf)
        for g in range(NG):
            blk = slice(g * NSH, (g + 1) * NSH)
            rows = slice(GB * g, GB * (g + 1))
            xc[0:GB, blk] = xh[rows]
            xc[GB : 2 * GB, blk] = xl[rows]
            xc[2 * GB : 3 * GB, blk] = xh[rows]
            xc[3 * GB : 4 * GB, blk] = msqh[rows]
            xc[4 * GB : 5 * GB, blk] = msql[rows]
        in_maps.append(
            {
                "xpk": xpk,
                "xc": np.ascontiguousarray(xc),
                "lhsTa": lhsTa,
                "lhsTw": lhsTw,
                "ebias": ebias,
            }
        )
    return in_maps


def _gather(results):
    """[NG, 128, NSH] per core, rows m=d*16+b  ->  full [B, N, D]."""
    outs = []
    for r in results:
        o = r["out"].astype(np.float32).reshape(NG, D, GB, NSH)  # [g, d, b, n]
        outs.append(o.transpose(0, 2, 3, 1).reshape(B, NSH, D))  # [b, n, d]
    full = np.concatenate(outs, axis=1)  # [B, N, D]
    return np.ascontiguousarray(full)


_NC_CACHE = {}


def run(inputs, trace=False, **trace_kwargs):
    """Builds (cached), runs on 8 cores, returns ((result, zeros), results)."""
    key = ("v3",)
    if key not in _NC_CACHE:
        _NC_CACHE[key] = _build()
    nc = _NC_CACHE[key]
    in_maps = _host_inputs(inputs["weights"], inputs["positions"])
    br = run_bass_kernel_spmd(
        nc, in_maps, list(range(NCORES)), trace=trace, **trace_kwargs
    )
    result = _gather(br.results)
    return (result, np.zeros_like(result)), br


def kernel(weights, weights_std, positions):
    out, _ = run(
        {"weights": weights, "weights_std": weights_std, "positions": positions}
    )
    return out
